# revision 28
# baseline (speedup 1.0000x reference)
"""DeepSeek-style MLA transformer block on 8 Trainium2 NeuronCores.

Two-collective design (collectives dominate wall time in this environment,
so everything after attention is token-sharded with full weights streamed):

  Stage A (token-sharded, 256 tok/core): attn_norm -> wq_a -> q_norm and
    wkv_a -> kv_norm + rope(k_pe); ONE AllGather of the bf16 latents
    [kv(512) | k_pe(64) | q(1536)] = [2112, 256] per core.
  Stage B (head-sharded, 2 heads/core): expand k_nope/v from the gathered
    kv latent, wq_b (+rope q), causal attention for the 2 local heads over
    all tokens; ONE AllToAll redistributes y from head-sharded to
    token-sharded ([2048 hv, 256 tok] per core, bf16).
  Stage C (token-sharded, collective-free): full wo (bf16, streamed) +
    residual -> res1 [2048, 256]; local ffn-norm; full SwiGLU MLP (bf16
    weights streamed, ~100MB/core, hidden under the MLP matmuls) + residual
    -> out [2048, 256]; host concatenates token shards.

All rmsnorm weights are folded into the following weight matrix on the host
(exact); softmax scale (-96) is folded into q at the wq_b eviction; rmsnorm
reduce+broadcast is an all-ones matmul (bf16 squares).

Precision: fp32 DMA + f32r matmuls for stage A and the q/k nope path;
bf16 for the latent exchange, v/p/y, wo and the MLP. Softmax and all psum
accumulation stay fp32.  Measured rel err ~5e-3 (gate 2e-2).
"""

import os
import sys

sys.path.insert(0, "/opt/trn_rl_repo")

from contextlib import ExitStack

import ml_dtypes
import numpy as np

import concourse.bacc as bacc
import concourse.bass as bass
import concourse.mybir as mybir
import concourse.tile as tile
from concourse.bass_utils import run_bass_kernel_spmd
from concourse.masks import make_identity

F32 = mybir.dt.float32
F32R = mybir.dt.float32r
BF16 = mybir.dt.bfloat16
F16 = mybir.dt.float16
AX = mybir.AxisListType.X
ADD = mybir.AluOpType.add
SUB = mybir.AluOpType.subtract
MUL = mybir.AluOpType.mult
AF = mybir.ActivationFunctionType

NCORES = 8
B, T, D = 2, 1024, 2048
H = 16
NOPE, ROPE = 128, 64
QKD = NOPE + ROPE  # 192
QLORA, KVLORA = 1536, 512
VHD = 128
INTER = 8192
EPS = 1e-6
SOFTSCALE = float(QKD) * -0.5  # -96.0

N_TOK = B * T  # 2048
S = N_TOK // NCORES  # 256 tokens per core
HPC = H // NCORES  # 2 heads per core
AGKV = KVLORA + ROPE  # 576
AGL = AGKV + QLORA  # 2112 merged latent rows


def _cfg(name, default="1"):
    return os.environ.get(name, default) == "1"


_CACHED_NC = None
LAST_RESULTS = None  # test.py reads these
LAST_IN_MAPS = None


def _rms_inv(nc, ones128, eps_ap, psum_pool, work_pool, chunks, dim, n, tag):
    """chunks: list of APs [128, n] covering `dim` feature rows (feature-major).
    Returns an SBUF tile [128, n] whose every row is 1/sqrt(mean_sq + eps).
    The all-ones reduce matmul runs in bf16 (1 cycle/row at any n)."""
    ss = psum_pool.tile([128, n], F32, tag="rms_ss", name=f"{tag}_ss")
    nchunks = len(chunks)
    for i, xc in enumerate(chunks):
        xx = work_pool.tile([128, n], F16, tag="rms_xx", name=f"{tag}_xx")
        nc.scalar.square(xx[:], xc)
        nc.tensor.matmul(
            ss[:], ones128[:], xx[:], start=(i == 0), stop=(i == nchunks - 1)
        )
    std = work_pool.tile([128, n], F32, tag="rms_std", name=f"{tag}_std")
    nc.scalar.activation(std[:], ss[:], AF.Sqrt, bias=eps_ap, scale=1.0 / dim)
    inv = work_pool.tile([128, n], F32, tag=f"{tag}_inv", name=f"{tag}_inv")
    nc.vector.reciprocal(inv[:], std[:])
    return inv


def _f32v(ap):
    """View a (possibly float32r) AP as plain float32 for DVE/ACT reads."""
    return ap.bitcast(F32) if ap.dtype == F32R else ap


def _rope(nc, pool, out64, in64, cs_a, cs_b, base, n, tag):
    """out64/in64: APs [64, n]; rows 0:32 = even lanes, 32:64 = odd lanes.
    in64 sits at partition base `base` (0 or 64).  cs_a/cs_b are cos/sin
    tables replicated so a slice at `base` lines up with in64 (walrus needs
    equal partition bases for 2-input SBUF ops): cs_a rows = cos|sin (per 32),
    cs_b rows = sin|cos."""
    in64 = _f32v(in64)
    t0 = pool.tile([32, n], F32, tag="rope_t0", name=f"{tag}_t0")
    t1 = pool.tile([32, n], F32, tag="rope_t1", name=f"{tag}_t1")
    nc.vector.tensor_tensor(t0[:], in64[0:32, :], cs_a[base : base + 32, :], MUL)
    nc.vector.tensor_tensor(t1[:], in64[32:64, :], cs_a[base + 32 : base + 64, :], MUL)
    nc.vector.tensor_tensor(out64[0:32, :], t0[:], t1[:], SUB)
    t2 = pool.tile([32, n], F32, tag="rope_t2", name=f"{tag}_t2")
    t3 = pool.tile([32, n], F32, tag="rope_t3", name=f"{tag}_t3")
    nc.vector.tensor_tensor(t2[:], in64[0:32, :], cs_b[base : base + 32, :], MUL)
    nc.vector.tensor_tensor(t3[:], in64[32:64, :], cs_b[base + 32 : base + 64, :], MUL)
    nc.vector.tensor_tensor(out64[32:64, :], t2[:], t3[:], ADD)


def _build():
    nc = bacc.Bacc("TRN2", target_bir_lowering=False, debug=False, num_devices=NCORES)
    RG = [list(range(NCORES))]
    r_attn = _cfg("R_ATTN")
    DT_AT = F32R if r_attn else F32

    def bc(ap, dt):
        return ap.bitcast(dt) if dt == F32R else ap

    def _coll(kind, op, ins, outs, group="X"):
        if os.environ.get("SKIP_COLL") == "1":
            return
        if group in os.environ.get("SKIP_GROUPS", ""):
            return
        nc.gpsimd.collective_compute(kind, op, replica_groups=RG, ins=ins, outs=outs)

    # ---- kernel parameters (per-core data supplied via in_maps) ----
    p_xs = nc.declare_dram_parameter("xs", [D, S], F32, isOutput=False)
    p_mask = nc.declare_dram_parameter("mask", [T, T], F32, isOutput=False)
    p_cosq = nc.declare_dram_parameter("cosq", [32, T], F32, isOutput=False)
    p_sinq = nc.declare_dram_parameter("sinq", [32, T], F32, isOutput=False)
    p_cosk = nc.declare_dram_parameter("cosk", [32, S], F32, isOutput=False)
    p_sink = nc.declare_dram_parameter("sink", [32, S], F32, isOutput=False)
    p_wqa = nc.declare_dram_parameter("wqaT", [D, QLORA], F32, isOutput=False)
    p_wkva = nc.declare_dram_parameter("wkvaT", [D, AGKV], F32, isOutput=False)
    p_wqb = nc.declare_dram_parameter("wqbT", [QLORA, HPC * QKD], F16, isOutput=False)
    p_wkb = nc.declare_dram_parameter("wkbT", [KVLORA, HPC * NOPE], F16, isOutput=False)
    p_wv = nc.declare_dram_parameter("wvT", [KVLORA, HPC * VHD], F16, isOutput=False)
    p_wof = nc.declare_dram_parameter("wof", [H * VHD, D], F16, isOutput=False)
    p_w13 = nc.declare_dram_parameter("w13f", [D, 2 * INTER], F16, isOutput=False)
    p_w2 = nc.declare_dram_parameter("w2f", [INTER, D], F16, isOutput=False)
    p_out = nc.declare_dram_parameter("out", [D, S], F32, isOutput=True)

    with tile.TileContext(
        nc, trace_sim=bool(os.environ.get("TILE_TRACE_SIM"))
    ) as tc, ExitStack() as root:
        dram = root.enter_context(tc.tile_pool(name="dram", bufs=1, space="DRAM"))
        agin = dram.tile([AGL, S], F16, name="agin")
        agout = dram.tile([NCORES * AGL, S], F16, addr_space="Shared", name="agout")
        a2a_in = dram.tile([H * VHD, S], F16, name="a2a_in")
        a2a_out = dram.tile([H * VHD, S], F16, name="a2a_out")

        const = root.enter_context(tc.tile_pool(name="const", bufs=1))
        ones128 = const.tile([128, 128], F16, name="ones128")
        nc.vector.memset(ones128[:], 1.0)
        eps_sb = const.tile([128, 1], F32, name="eps_sb")
        nc.vector.memset(eps_sb[:], EPS)
        identb = const.tile([128, 128], F16, name="identb")
        make_identity(nc, identb[:])
        # cos/sin tables replicated per 32 rows so any 64-aligned rope input
        # base finds matching-base cos and sin slices (see _rope)
        csq_a = const.tile([128, T], F32, name="csq_a")
        csq_b = const.tile([128, T], F32, name="csq_b")
        for r in range(4):
            nc.sync.dma_start(
                csq_a[32 * r : 32 * (r + 1), :], (p_cosq if r % 2 == 0 else p_sinq)[:, :]
            )
            nc.sync.dma_start(
                csq_b[32 * r : 32 * (r + 1), :], (p_sinq if r % 2 == 0 else p_cosq)[:, :]
            )

        # x (feature-major, local tokens) and res1 live across all stages
        xres = root.enter_context(tc.tile_pool(name="xres", bufs=1))
        x_sb = xres.tile([128, D // 128, S], F32, name="x_sb")
        nc.sync.dma_start(x_sb[:], p_xs.rearrange("(ko p) n -> p ko n", p=128))
        res1_sb = xres.tile([128, D // 128, S], F32, name="res1_sb")
        h2_sb = xres.tile([128, D // 128, S], F16, name="h2_sb")

        # ---- stage-B weight prefetch (no deps -> DMAs overlap stage A) ----
        w_pre_cm = tc.tile_pool(name="w_pre", bufs=1)
        w_pre = w_pre_cm.__enter__()
        wkb_sb = w_pre.tile([128, 4, HPC * NOPE], F16, name="wkb_sb")
        nc.sync.dma_start(wkb_sb[:], p_wkb.rearrange("(ko p) m -> p ko m", p=128))
        wv_sb = w_pre.tile([128, 4, HPC * VHD], F16, name="wv_sb")
        nc.sync.dma_start(wv_sb[:], p_wv.rearrange("(ko p) m -> p ko m", p=128))
        wqb_sb = w_pre.tile([128, QLORA // 128, HPC * QKD], F16, name="wqb_sb")
        nc.sync.dma_start(wqb_sb[:], p_wqb.rearrange("(ko p) m -> p ko m", p=128))

        # =========================== Stage A ===========================
        with ExitStack() as sa:
            a_res = sa.enter_context(tc.tile_pool(name="a_res", bufs=1))
            a_wk = sa.enter_context(tc.tile_pool(name="a_wk", bufs=3))
            a_w = sa.enter_context(tc.tile_pool(name="a_w", bufs=4))
            a_ps = sa.enter_context(tc.tile_pool(name="a_ps", bufs=2, space="PSUM"))

            inv_a = _rms_inv(
                nc, ones128, eps_sb[:], a_ps, a_wk,
                [x_sb[:, k, :] for k in range(D // 128)], D, S, "rmsa",
            )
            xh = a_res.tile([128, D // 128, S], DT_AT, name="xh")
            for k in range(D // 128):
                nc.vector.tensor_tensor(xh[:, k, :], x_sb[:, k, :], inv_a[:], MUL)

            # kv_all = wkv_a' @ xh -> [576, S]; rows 0:512 latent, 512:576 rope key
            # weights stream as [128, 384/192] slabs (wide rows -> full DMA bw)
            kv = a_res.tile([128, 5, S], F32, name="kv")  # 4x128 + 1x64 (in row 4)
            for goff, gsz, msizes in ((0, 384, (128, 128, 128)), (384, 192, (128, 64))):
                pss = [
                    a_ps.tile([128, S], F32, tag=f"a_mm{i}", name=f"kv_ps{goff}_{i}")
                    for i in range(len(msizes))
                ]
                for k in range(D // 128):
                    wt = a_w.tile([128, 384], DT_AT, tag="wkva_t", name="wkva_t")
                    nc.sync.dma_start(
                        wt[:, :gsz],
                        bc(p_wkva[128 * k : 128 * (k + 1), goff : goff + gsz], DT_AT),
                    )
                    off = 0
                    for i, msz in enumerate(msizes):
                        nc.tensor.matmul(
                            pss[i][:msz, :], wt[:, off : off + msz], xh[:, k, :],
                            start=(k == 0), stop=(k == D // 128 - 1),
                        )
                        off += msz
                off = 0
                for i, msz in enumerate(msizes):
                    nc.any.tensor_copy(kv[:msz, (goff + off) // 128, :], pss[i][:msz, :])
                    off += msz
            inv_kv = _rms_inv(
                nc, ones128, eps_sb[:], a_ps, a_wk,
                [kv[:, m, :] for m in range(4)], KVLORA, S, "rmskv",
            )
            for m in range(4):
                ln = a_wk.tile([128, S], F16, tag="latn", name="latn")
                nc.vector.tensor_tensor(ln[:], kv[:, m, :], inv_kv[:], MUL)
                nc.sync.dma_start(agin[128 * m : 128 * (m + 1), :], ln[:])
            # rope the decoupled key (shared across heads), feature-permuted on host
            csk_a = a_res.tile([64, S], F32, name="csk_a")
            csk_b = a_res.tile([64, S], F32, name="csk_b")
            nc.sync.dma_start(csk_a[0:32, :], p_cosk[:, :])
            nc.sync.dma_start(csk_a[32:64, :], p_sink[:, :])
            nc.sync.dma_start(csk_b[0:32, :], p_sink[:, :])
            nc.sync.dma_start(csk_b[32:64, :], p_cosk[:, :])
            kpe_r = a_wk.tile([64, S], F16, tag="kpe_r", name="kpe_r")
            _rope(nc, a_wk, kpe_r[:], kv[0:64, 4, :], csk_a[:], csk_b[:], 0, S, "ropek")
            nc.sync.dma_start(agin[KVLORA : KVLORA + ROPE, :], kpe_r[:])
            # q_lat = wq_a' @ xh   -> [1536, S] feature-major
            ql = a_res.tile([128, QLORA // 128, S], F32, name="ql")
            for g in range(QLORA // 384):
                pss = [
                    a_ps.tile([128, S], F32, tag=f"a_mm{i}", name=f"ql_ps{g}_{i}")
                    for i in range(3)
                ]
                for k in range(D // 128):
                    wt = a_w.tile([128, 384], DT_AT, tag="wqa_t", name="wqa_t")
                    nc.sync.dma_start(
                        wt[:],
                        bc(p_wqa[128 * k : 128 * (k + 1), 384 * g : 384 * (g + 1)], DT_AT),
                    )
                    for i in range(3):
                        nc.tensor.matmul(
                            pss[i][:], wt[:, 128 * i : 128 * (i + 1)], xh[:, k, :],
                            start=(k == 0), stop=(k == D // 128 - 1),
                        )
                for i in range(3):
                    nc.any.tensor_copy(ql[:, 3 * g + i, :], pss[i][:])
            inv_q = _rms_inv(
                nc, ones128, eps_sb[:], a_ps, a_wk,
                [ql[:, m, :] for m in range(QLORA // 128)], QLORA, S, "rmsq",
            )
            for m in range(QLORA // 128):
                qh = a_wk.tile([128, S], F16, tag="qh", name="qh")
                nc.vector.tensor_tensor(qh[:], ql[:, m, :], inv_q[:], MUL)
                nc.sync.dma_start(agin[AGKV + 128 * m : AGKV + 128 * (m + 1), :], qh[:])

        _coll("AllGather", mybir.AluOpType.bypass, [agin[:].opt()], [agout[:].opt()],
              group="L")

        # =========================== Stage B ===========================
        with ExitStack() as sb:
            b_res = sb.enter_context(tc.tile_pool(name="b_res", bufs=1))
            b_wk = sb.enter_context(tc.tile_pool(name="b_wk", bufs=2))

            k_sb = b_res.tile([128, HPC, N_TOK], DT_AT, name="k_sb")
            kpe_sb = b_res.tile([128, N_TOK], F16, name="kpe_sb")
            v_sb = b_res.tile([128, N_TOK // 128, HPC * VHD], F16, name="v_sb")
            q_sb = b_res.tile([128, 3, N_TOK], DT_AT, name="q_sb")
            qpe_sb = b_res.tile([128, N_TOK], F16, name="qpe_sb")
            y_sb = b_res.tile([128, HPC, N_TOK], F16, name="y_sb")

            # ---- expand k_nope and v (token-major) for the 2 local heads ----
            with ExitStack() as s1:
                b_rhs1 = s1.enter_context(tc.tile_pool(name="b_rhs1", bufs=8))
                b1_ps = s1.enter_context(
                    tc.tile_pool(name="b1_ps", bufs=2, space="PSUM")
                )
                for blk in range(NCORES):
                    base = AGL * blk
                    lat = [
                        b_rhs1.tile([128, S], F16, tag="lat", name="lat")
                        for _ in range(4)
                    ]
                    for k in range(4):
                        nc.sync.dma_start(
                            lat[k][:], agout[base + 128 * k : base + 128 * (k + 1), :]
                        )
                    for m in range(HPC):
                        psk = b1_ps.tile([128, S], F32, tag="psk", name="psk")
                        for k in range(4):
                            nc.tensor.matmul(
                                psk[:], wkb_sb[:, k, 128 * m : 128 * (m + 1)],
                                lat[k][:], start=(k == 0), stop=(k == 3),
                            )
                        nc.any.tensor_copy(k_sb[:, m, S * blk : S * (blk + 1)], psk[:])
                    for th in range(S // 128):
                        psv = b1_ps.tile([128, HPC * VHD], F32, tag="psv", name="psv")
                        for k in range(4):
                            nc.tensor.matmul(
                                psv[:], lat[k][:, 128 * th : 128 * (th + 1)],
                                wv_sb[:, k, :], start=(k == 0), stop=(k == 3),
                            )
                        nc.any.tensor_copy(v_sb[:, (S // 128) * blk + th, :], psv[:])
                    nc.sync.dma_start(
                        kpe_sb[0:64, S * blk : S * (blk + 1)],
                        agout[base + KVLORA : base + KVLORA + ROPE, :],
                    )
                    nc.sync.dma_start(
                        kpe_sb[64:128, S * blk : S * (blk + 1)],
                        agout[base + KVLORA : base + KVLORA + ROPE, :],
                    )

            # ---- q for the 2 local heads (fold SOFTSCALE here) ----
            with ExitStack() as s2:
                b_rhs2 = s2.enter_context(tc.tile_pool(name="b_rhs2", bufs=8))
                b2_ps = s2.enter_context(
                    tc.tile_pool(name="b2_ps", bufs=2, space="PSUM")
                )
                for blk in range(NCORES):
                    base = AGL * blk + AGKV
                    qrhs = [
                        b_rhs2.tile([128, S], F16, tag="qrhs", name="qrhs")
                        for _ in range(QLORA // 128)
                    ]
                    for k in range(QLORA // 128):
                        nc.sync.dma_start(
                            qrhs[k][:],
                            agout[base + 128 * k : base + 128 * (k + 1), :],
                        )
                    psq = [
                        b2_ps.tile([128, S], F32, tag=f"psq{m}", name=f"psq{m}")
                        for m in range(3)
                    ]
                    for k in range(QLORA // 128):
                        for m in range(3):
                            nc.tensor.matmul(
                                psq[m][:], wqb_sb[:, k, 128 * m : 128 * (m + 1)],
                                qrhs[k][:],
                                start=(k == 0), stop=(k == QLORA // 128 - 1),
                            )
                    for m in range(3):
                        nc.scalar.mul(
                            q_sb[:, m, S * blk : S * (blk + 1)], psq[m][:], SOFTSCALE
                        )

            # rope q_pe: q_sb chunk 2 = [h0_e, h0_o, h1_e, h1_o] x32 rows
            with tc.tile_pool(name="rope_wk", bufs=1) as rp:
                for b in range(B):
                    for h in range(HPC):
                        _rope(
                            nc, rp,
                            qpe_sb[64 * h : 64 * (h + 1), T * b : T * (b + 1)],
                            q_sb[64 * h : 64 * (h + 1), 2, T * b : T * (b + 1)],
                            csq_a[:, :], csq_b[:, :],
                            64 * h, T, f"ropeq{b}{h}",
                        )

            # ---- attention ----
            with ExitStack() as s3:
                b3_ps = s3.enter_context(
                    tc.tile_pool(name="b3_ps", bufs=2, space="PSUM")
                )
                for h in range(HPC):
                    for sc in range(T // 128):
                        W = 512 * (sc // 4 + 1)
                        mask_sb = b_wk.tile([128, T], F32, tag="mask_sb", name="mask_sb")
                        nc.sync.dma_start(
                            mask_sb[:, :W], p_mask[128 * sc : 128 * (sc + 1), :W]
                        )
                        for b in range(B):
                            s0 = T * b + 128 * sc
                            scs = b_wk.tile([128, T], F32, tag="scs", name="scs")
                            for tcx in range(W // 512):
                                t0 = T * b + 512 * tcx
                                ps = b3_ps.tile([128, 512], F32, tag="ps_qk", name="ps_qk")
                                nc.tensor.matmul(
                                    ps[:],
                                    q_sb[:, h, s0 : s0 + 128],
                                    k_sb[:, h, t0 : t0 + 512],
                                    start=True, stop=False,
                                )
                                nc.tensor.matmul(
                                    ps[:],
                                    qpe_sb[64 * h : 64 * (h + 1), s0 : s0 + 128],
                                    kpe_sb[64 * h : 64 * (h + 1), t0 : t0 + 512],
                                    start=False, stop=True,
                                )
                                nc.vector.tensor_tensor(
                                    scs[:, 512 * tcx : 512 * (tcx + 1)], ps[:],
                                    mask_sb[:, 512 * tcx : 512 * (tcx + 1)], ADD,
                                )
                            nmax = b_wk.tile([128, 1], F32, tag="nmax", name="nmax")
                            nc.vector.reduce_max(
                                nmax[:], scs[:, :W], axis=AX, negate=True
                            )
                            p_sb = b_wk.tile([128, T], F16, tag="p_sb", name="p_sb")
                            zsum = b_wk.tile([128, 1], F32, tag="zsum", name="zsum")
                            nc.scalar.activation(
                                p_sb[:, :W], scs[:, :W], AF.Exp,
                                bias=nmax[:], accum_out=zsum[:],
                            )
                            invz = b_wk.tile([128, 1], F32, tag="invz", name="invz")
                            nc.vector.reciprocal(invz[:], zsum[:])
                            nc.vector.tensor_scalar_mul(
                                p_sb[:, :W], p_sb[:, :W], invz[:]
                            )
                            ptr = b_wk.tile([128, T], F16, tag="ptr", name="ptr")
                            for tj in range(W // 128):
                                pst = b3_ps.tile([128, 128], F16, tag="pst", name="pst")
                                nc.tensor.transpose(
                                    pst[:], p_sb[:, 128 * tj : 128 * (tj + 1)], identb[:]
                                )
                                nc.any.tensor_copy(
                                    ptr[:, 128 * tj : 128 * (tj + 1)], pst[:]
                                )
                            psy = b3_ps.tile([128, 128], F32, tag="psy", name="psy")
                            ntj = W // 128
                            for tj in range(ntj):
                                nc.tensor.matmul(
                                    psy[:],
                                    v_sb[:, (T // 128) * b + tj, VHD * h : VHD * (h + 1)],
                                    ptr[:, 128 * tj : 128 * (tj + 1)],
                                    start=(tj == 0), stop=(tj == ntj - 1),
                                )
                            nc.any.tensor_copy(y_sb[:, h, s0 : s0 + 128], psy[:])

            # y: head-sharded -> token-sharded via one AllToAll.
            # block j (rows 256j:256(j+1)) goes to core j = our 2 heads for
            # core j's tokens; received block i = core i's heads (2i, 2i+1)
            # for OUR tokens -> rows are global-head-major [2048, 256].
            for h in range(HPC):
                for j in range(NCORES):
                    nc.sync.dma_start(
                        a2a_in[S * j + VHD * h : S * j + VHD * (h + 1), :],
                        y_sb[:, h, S * j : S * (j + 1)],
                    )
            _coll("AllToAll", mybir.AluOpType.bypass, [a2a_in[:].opt()],
                  [a2a_out[:].opt()], group="Y")

        w_pre_cm.__exit__(None, None, None)

        # ====== Stage C (token-sharded, no collectives): wo + norm + MLP ======
        with ExitStack() as sc_stack:
            c_res = sc_stack.enter_context(tc.tile_pool(name="c_res", bufs=1))
            c_wk = sc_stack.enter_context(tc.tile_pool(name="c_wk", bufs=3))
            c_w = sc_stack.enter_context(tc.tile_pool(name="c_w", bufs=3))

            y2 = c_res.tile([128, H * VHD // 128, S], F16, name="y2")
            nc.sync.dma_start(y2[:], a2a_out[:].rearrange("(ko p) n -> p ko n", p=128))

            # wo (full, bf16, streamed as [128, 1024] k-slabs) + residual
            with tc.tile_pool(name="wo_ps", bufs=1, space="PSUM") as wo_ps:
                for mg in range(2):
                    pso = [
                        wo_ps.tile([128, S], F32, tag=f"pso{m}", name=f"pso{m}")
                        for m in range(8)
                    ]
                    for k in range(H * VHD // 128):
                        wok = c_w.tile([128, 1024], F16, tag="wok", name="wok")
                        nc.sync.dma_start(
                            wok[:],
                            p_wof[128 * k : 128 * (k + 1),
                                  1024 * mg : 1024 * (mg + 1)],
                        )
                        for m in range(8):
                            nc.tensor.matmul(
                                pso[m][:], wok[:, 128 * m : 128 * (m + 1)],
                                y2[:, k, :],
                                start=(k == 0), stop=(k == H * VHD // 128 - 1),
                            )
                    for m in range(8):
                        mm = 8 * mg + m
                        nc.vector.tensor_tensor(
                            res1_sb[:, mm, :], pso[m][:], x_sb[:, mm, :], ADD
                        )

            # local ffn-norm -> h2 (bf16 for the MLP matmuls)
            with tc.tile_pool(name="n_ps", bufs=1, space="PSUM") as n_ps:
                inv_f = _rms_inv(
                    nc, ones128, eps_sb[:], n_ps, c_wk,
                    [res1_sb[:, k, :] for k in range(D // 128)], D, S, "rmsf",
                )
                for k in range(D // 128):
                    nc.vector.tensor_tensor(
                        h2_sb[:, k, :], res1_sb[:, k, :], inv_f[:], MUL
                    )

        # ============== MLP (full weights, bf16, token-sharded) ==============
        with ExitStack() as sm:
            m_res = sm.enter_context(tc.tile_pool(name="m_res", bufs=1))
            m_w = sm.enter_context(tc.tile_pool(name="m_w", bufs=3))
            m_wk = sm.enter_context(tc.tile_pool(name="m_wk", bufs=3))

            g_sb = m_res.tile([128, INTER // 128, S], F16, name="g_sb")
            with tc.tile_pool(name="m_ps1", bufs=2, space="PSUM") as m_ps1:
                for j in range(INTER // 128):
                    wj = m_w.tile([128, D // 128, 256], F16, tag="wj", name="wj")
                    nc.sync.dma_start(
                        wj[:],
                        p_w13[:, 256 * j : 256 * (j + 1)].rearrange(
                            "(ko p) m -> p ko m", p=128
                        ),
                    )
                    psa = m_ps1.tile([128, S], F32, tag="psa", name="psa")
                    psb = m_ps1.tile([128, S], F32, tag="psb", name="psb")
                    for k in range(D // 128):
                        nc.tensor.matmul(
                            psa[:], wj[:, k, 0:128], h2_sb[:, k, :],
                            start=(k == 0), stop=(k == D // 128 - 1),
                        )
                        nc.tensor.matmul(
                            psb[:], wj[:, k, 128:256], h2_sb[:, k, :],
                            start=(k == 0), stop=(k == D // 128 - 1),
                        )
                    tsi = m_wk.tile([128, S], F32, tag="tsi", name="tsi")
                    nc.scalar.activation(tsi[:], psa[:], AF.Silu)
                    nc.vector.tensor_tensor(g_sb[:, j, :], tsi[:], psb[:], MUL)

            with tc.tile_pool(name="m_ps2", bufs=1, space="PSUM") as m_ps2:
                for mg in range(2):
                    ps2 = [
                        m_ps2.tile([128, S], F32, tag=f"ps2_{m}", name=f"ps2_{m}")
                        for m in range(8)
                    ]
                    for k in range(INTER // 128):
                        w2k = m_w.tile([128, 1024], F16, tag="w2k", name="w2k")
                        nc.sync.dma_start(
                            w2k[:],
                            p_w2[128 * k : 128 * (k + 1),
                                 1024 * mg : 1024 * (mg + 1)],
                        )
                        for m in range(8):
                            nc.tensor.matmul(
                                ps2[m][:], w2k[:, 128 * m : 128 * (m + 1)],
                                g_sb[:, k, :],
                                start=(k == 0), stop=(k == INTER // 128 - 1),
                            )
                    for m in range(8):
                        mm = 8 * mg + m
                        ot = m_wk.tile([128, S], F32, tag="ot", name="ot")
                        nc.vector.tensor_tensor(
                            ot[:], ps2[m][:], res1_sb[:, mm, :], ADD
                        )
                        nc.sync.dma_start(p_out[128 * mm : 128 * (mm + 1), :], ot[:])

    nc.compile()
    return nc


def _rope_perm(n):
    """Permutation putting even lanes first then odd lanes, for an n-row rope
    block (n even): [0,2,4,...,n-2, 1,3,5,...,n-1]."""
    return np.concatenate([np.arange(0, n, 2), np.arange(1, n, 2)])


def kernel(**inputs):
    global _CACHED_NC, LAST_RESULTS, LAST_IN_MAPS
    f32 = lambda a: np.ascontiguousarray(np.asarray(a), dtype=np.float32)
    f16 = lambda a: np.ascontiguousarray(np.asarray(a, dtype=np.float16))

    x = f32(inputs["x"]).reshape(N_TOK, D)
    mask = f32(inputs["mask"])
    cos = f32(inputs["freqs_cos"])  # [T, 32]
    sin = f32(inputs["freqs_sin"])
    attn_nw = f32(inputs["attn_norm_w"])
    wq_a = f32(inputs["wq_a"]) * attn_nw[None, :]
    q_nw = f32(inputs["q_norm_w"])
    wq_b = f32(inputs["wq_b"]) * q_nw[None, :]
    wkv_a = f32(inputs["wkv_a"]) * attn_nw[None, :]
    kv_nw = f32(inputs["kv_norm_w"])
    wkv_b = f32(inputs["wkv_b"]) * kv_nw[None, :]
    wo = f32(inputs["wo"])
    ffn_nw = f32(inputs["ffn_norm_w"])
    w1 = f32(inputs["w1"]) * ffn_nw[None, :]
    w3 = f32(inputs["w3"]) * ffn_nw[None, :]
    w2 = f32(inputs["w2"])

    xT = np.ascontiguousarray(x.T)  # [D, N_TOK] feature-major
    cosT = np.ascontiguousarray(cos.T)  # [32, T]
    sinT = np.ascontiguousarray(sin.T)

    # wkv_a rows: keep 0:512 (latent); permute rope rows 512:576 to even|odd
    pk = _rope_perm(ROPE)
    wkva_p = wkv_a.copy()
    wkva_p[KVLORA:] = wkv_a[KVLORA:][pk]
    wkvaT = np.ascontiguousarray(wkva_p.T)  # [D, 576]

    wqaT = np.ascontiguousarray(wq_a.T)  # [D, 1536]

    # wo: y arrives as [h*VHD + d] rows in natural global-head order
    wof = f16(wo.T)  # [H*VHD, D]

    # w13: columns interleaved [w1_j(128) | w3_j(128)] per 256-col group
    w13f = np.empty((D, 2 * INTER), np.float32)
    for j in range(INTER // 128):
        w13f[:, 256 * j : 256 * j + 128] = w1[128 * j : 128 * (j + 1)].T
        w13f[:, 256 * j + 128 : 256 * (j + 1)] = w3[128 * j : 128 * (j + 1)].T
    w13f = f16(w13f)
    w2f = f16(w2.T)  # [INTER, D]

    in_maps = []
    for c in range(NCORES):
        heads = [HPC * c + j for j in range(HPC)]
        # wq_b rows per head h: h*QKD .. h*QKD+192 (128 nope + 64 rope)
        # target col order: [h0_nope(128), h1_nope(128), h0_rope_eo(64), h1_rope_eo(64)]
        cols = []
        for h in heads:
            cols.append(wq_b[h * QKD : h * QKD + NOPE])
        for h in heads:
            cols.append(wq_b[h * QKD + NOPE : (h + 1) * QKD][pk])
        wqbT = f16(np.concatenate(cols, axis=0).T)  # [1536, 384]

        # wkv_b rows per head h: h*(NOPE+VHD) + [0:128]=k_nope, [128:256]=v
        kw = np.concatenate(
            [wkv_b[h * (NOPE + VHD) : h * (NOPE + VHD) + NOPE] for h in heads], axis=0
        )
        vw = np.concatenate(
            [wkv_b[h * (NOPE + VHD) + NOPE : (h + 1) * (NOPE + VHD)] for h in heads],
            axis=0,
        )
        wkbT = f16(kw.T)  # [512, 256]
        wvT = f16(vw.T)  # [512, 256]

        tpos = (S * c) % T  # position within batch of this token shard
        in_maps.append(
            {
                "xs": np.ascontiguousarray(xT[:, S * c : S * (c + 1)]),
                "mask": mask,
                "cosq": cosT,
                "sinq": sinT,
                "cosk": np.ascontiguousarray(cosT[:, tpos : tpos + S]),
                "sink": np.ascontiguousarray(sinT[:, tpos : tpos + S]),
                "wqaT": wqaT,
                "wkvaT": wkvaT,
                "wqbT": wqbT,
                "wkbT": wkbT,
                "wvT": wvT,
                "wof": wof,
                "w13f": w13f,
                "w2f": w2f,
            }
        )

    LAST_IN_MAPS = in_maps
    if _CACHED_NC is None:
        _CACHED_NC = _build()
    nc = _CACHED_NC

    trace = bool(os.environ.get("KERNEL_TRACE"))
    res = run_bass_kernel_spmd(
        nc, in_maps, core_ids=list(range(NCORES)), trace=trace
    )
    LAST_RESULTS = res

    outT = np.concatenate([res.results[c]["out"] for c in range(NCORES)], axis=1)
    return np.ascontiguousarray(outT.T).reshape(B, T, D).astype(np.float32)
